# revision 13
# baseline (speedup 1.0000x reference)
"""Conv2d(1->16,5x5,p2) + BN(inference) + ReLU + MaxPool2d(2) on 8 NeuronCores.

The end-to-end call is bound by the axon tunnel (~30-40MB/s, full duplex), so
the design minimizes bytes on the wire and overlaps upload with download:
  - x is padded + cast to fp16 on the host (13.3MB up instead of 26.6MB f32).
  - Conv weights (BN-folded, Toeplitz lhsT) are fp16, uploaded once and cached
    on-device across calls.
  - The kernel quantizes each output slab to uint8 with a per-partition scale
    (the slab max, computed on-device). Scale f32 bytes are bitcast into the
    tail of the same u8 output buffer, so each chunk returns ONE tensor:
    ~6.4MB down per chunk instead of 12.9MB f32.
  - The batch is split into C=4 chunks pipelined through a cached jit: chunk
    n+1 uploads while chunk n downloads (full-duplex tunnel), fetches run on
    a 2-thread pool, dequantization overlaps the remaining transfers.
  - Donated output buffers are created on-device by a cached zeros jit, not
    uploaded from the host.

Device strategy (per core, data parallelism over batch):
  - BN is folded into the conv weights/bias on the host.
  - Conv is computed on the TensorEngine as a single matmul per 16-output-row
    slab: contraction K = (dx-block j in 0..4) x (input row yi in 0..19) = 100.
    The 5 dx shifts are materialized as 5 partition-blocks of the slab tile,
    loaded directly from HBM with column offset j (overlapping reads).
    The dy taps are encoded in a Toeplitz weight matrix lhsT[(j,yi), (o,yp)].
  - Two matmuls per slab produce even / odd output rows in separate PSUM
    banks, so the 2x2 maxpool becomes: vertical max = elementwise max of the
    two PSUM tiles (DVE), horizontal max = strided max in SBUF, then the
    quantization tail: per-partition slab max -> reciprocal -> fused
    ReLU+bias+scale activation casting straight to uint8.
"""

import numpy as np
import queue
import threading
from concurrent.futures import ThreadPoolExecutor

import jax
import jax.numpy as jnp
from jax.experimental.shard_map import shard_map
from jax.sharding import Mesh, NamedSharding, PartitionSpec

import concourse.bass as bass
import concourse.bacc as bacc
import concourse.tile as tile
import concourse.mybir as mybir
from concourse import bass2jax

F32 = mybir.dt.float32
F16 = mybir.dt.float16
U8 = mybir.dt.uint8
I8 = mybir.dt.int8
N_CORES = 8
B, H, W = 128, 224, 224
PB = B // N_CORES          # images per core
NCHUNK = 8                 # pipeline chunks per call
PBC = PB // NCHUNK         # images per core per chunk
NP = PBC // 2              # image pairs per core per chunk
PH, PW = H + 4, W + 4      # host-padded image
OC = 16
HO, WO = H // 2, W // 2    # 112, 112
YB = 16                    # conv output rows per slab
NT = H // YB               # 14 slabs per image pair
KROWS = YB + 4             # input rows per dx-block
K = 5 * KROWS              # 100 contraction partitions
BN_EPS = 1e-5
QMAX = 254.5               # u8 quantization full-scale (0.5 headroom for +0.5 rounding bias)
QBYTES = PBC * OC * HO * WO          # q payload bytes per core
SBYTES = NP * NT * 128 * 4           # scale payload bytes per core
FLAT = QBYTES + SBYTES

_CACHE: dict = {}


def _build_nc():
    nc = bacc.Bacc("TRN2", num_devices=N_CORES)
    xpad = nc.dram_tensor("xpad", [PBC, PH, PW], I8, kind="ExternalInput")
    rsc_d = nc.dram_tensor("rsc", [NP * NT, K], F32, kind="ExternalInput")
    lhsE_d = nc.dram_tensor("lhsE", [K, 128], F16, kind="ExternalInput")
    lhsO_d = nc.dram_tensor("lhsO", [K, 128], F16, kind="ExternalInput")
    bias_d = nc.dram_tensor("bias", [128, 1], F32, kind="ExternalInput")
    outb = nc.dram_tensor("outb", [FLAT], U8, kind="ExternalOutput")

    with tile.TileContext(nc) as tc:
        with (
            tc.tile_pool(name="const", bufs=1) as constp,
            tc.tile_pool(name="s", bufs=4) as sp,
            tc.tile_pool(name="v", bufs=3) as vp,
            tc.tile_pool(name="h", bufs=3) as hp,
            tc.tile_pool(name="m", bufs=3) as mp,
            tc.tile_pool(name="q", bufs=3) as qp,
            tc.tile_pool(name="ps", bufs=4, space="PSUM") as pp,
        ):
            lE = constp.tile([K, 128], F16, tag="lE")
            nc.sync.dma_start(lE[:], lhsE_d.ap())
            lO = constp.tile([K, 128], F16, tag="lO")
            nc.sync.dma_start(lO[:], lhsO_d.ap())
            bt = constp.tile([128, 1], F32, tag="bias")
            nc.sync.dma_start(bt[:], bias_d.ap())
            # per-slab per-partition input dequant scales: RS[p, col]
            RS = constp.tile([K, NP * NT], F32, tag="RS")
            nc.sync.dma_start(RS[:], bass.AP(rsc_d, 0, [[1, K], [K, NP * NT]]))
            # slab maxes, one column per (pair, slab); DMA'd out once at the end
            SC = constp.tile([128, NP * NT], F32, tag="SC")

            for pi in range(NP):            # image pairs
                for t in range(NT):         # y slabs
                    y0 = YB * t
                    col = pi * NT + t
                    S8 = sp.tile([K, 448], I8, tag="S8")
                    for i in range(2):
                        src = bass.AP(
                            xpad,
                            (2 * pi + i) * PH * PW + y0 * PW,
                            [[1, 5], [PW, KROWS], [1, 224]],
                        )
                        nc.sync.dma_start(S8[:, i * 224:(i + 1) * 224], src)
                    # dequant int8 -> fp16 with the per-row scale (ACT copy)
                    S = sp.tile([K, 448], F16, tag="S")
                    nc.scalar.activation(
                        S[:], S8[:], mybir.ActivationFunctionType.Copy,
                        scale=RS[:, col:col + 1],
                    )

                    pe_t = pp.tile([128, 448], F32, tag="ps")
                    nc.tensor.matmul(pe_t[:], lE[:], S[:], start=True, stop=True)
                    po_t = pp.tile([128, 448], F32, tag="ps")
                    nc.tensor.matmul(po_t[:], lO[:], S[:], start=True, stop=True)

                    # ACT drains the odd bank to SBUF (DVE cannot read two
                    # PSUM streams in one tensor_tensor)
                    CO = vp.tile([128, 448], F32, tag="CO")
                    nc.scalar.copy(CO[:], po_t[:])
                    # vertical max: PSUM + SBUF operands
                    V = vp.tile([128, 448], F32, tag="V")
                    nc.vector.tensor_max(V[:], pe_t[:], CO[:])
                    # horizontal max: strided SBUF
                    Hm = hp.tile([128, 224], F32, tag="H")
                    v4 = V[:].rearrange("p (i xp two) -> p i xp two", i=2, two=2)
                    h3 = Hm[:].rearrange("p (i xp) -> p i xp", i=2)
                    nc.vector.tensor_max(h3, v4[:, :, :, 0], v4[:, :, :, 1])

                    # ---- u8 quantization tail ----
                    Mx = mp.tile([128, 1], F32, tag="Mx")
                    nc.vector.reduce_max(Mx[:], Hm[:], axis=mybir.AxisListType.X)
                    # slab max of relu(conv+bias) = relu(max(conv)+bias)
                    Mr = mp.tile([128, 1], F32, tag="Mr")
                    nc.scalar.activation(
                        Mr[:], Mx[:], mybir.ActivationFunctionType.Relu,
                        bias=bt[:, 0:1], scale=1.0,
                    )
                    Mc = SC[:, col:col + 1]
                    nc.vector.tensor_scalar_max(Mc, Mr[:], 1e-20)
                    R = mp.tile([128, 1], F32, tag="R")
                    nc.vector.reciprocal(R[:], Mc)
                    Sg = mp.tile([128, 1], F32, tag="Sg")
                    nc.scalar.mul(Sg[:], R[:], QMAX)          # Sg = QMAX / max
                    # Bq = Sg*bias + 0.5 (0.5 turns the u8 trunc-cast into rounding)
                    Bq = mp.tile([128, 1], F32, tag="Bq")
                    nc.vector.tensor_scalar(
                        Bq[:], bt[:, 0:1], Sg[:, 0:1], 0.0,
                        op0=mybir.AluOpType.mult, op1=mybir.AluOpType.add,
                    )
                    # q = relu(Sg*conv + Sg*bias + 0.5) cast to u8
                    Q = qp.tile([128, 224], U8, tag="Q")
                    nc.scalar.activation(
                        Q[:], Hm[:], mybir.ActivationFunctionType.Relu,
                        bias=Bq[:, 0:1], scale=Sg[:, 0:1],
                    )

                    for i in range(2):
                        dst = bass.AP(
                            outb,
                            (2 * pi + i) * OC * HO * WO + (8 * t) * WO,
                            [[HO * WO, OC], [WO, 8], [1, WO]],
                        )
                        nc.scalar.dma_start(dst, Q[:, i * WO:(i + 1) * WO])

            # scale bytes: byte addr = QBYTES + (col*128 + m)*4 + k
            scb = SC[:].bitcast(U8).rearrange("p (c k) -> p c k", k=4)
            dst = bass.AP(outb, QBYTES, [[4, 128], [512, NP * NT], [1, 4]])
            nc.sync.dma_start(dst, scb)

    nc.compile()
    return nc


def _host_weights(conv_w, conv_b, gamma, beta, run_mean, run_var):
    scale = (gamma / np.sqrt(run_var + BN_EPS)).astype(np.float32)
    wf = (conv_w[:, 0] * scale[:, None, None]).astype(np.float32)       # [16,5,5]
    bf = (conv_b * scale + beta - run_mean * scale).astype(np.float32)  # [16]

    lhsE = np.zeros((K, 128), np.float32)
    lhsO = np.zeros((K, 128), np.float32)
    bias = np.zeros((128, 1), np.float32)
    for o in range(OC):
        for yp in range(8):
            m = o * 8 + yp
            bias[m, 0] = bf[o]
            for j in range(5):
                for dy in range(5):
                    lhsE[j * KROWS + 2 * yp + dy, m] = wf[o, dy, j]
                    lhsO[j * KROWS + 2 * yp + 1 + dy, m] = wf[o, dy, j]
    return lhsE.astype(np.float16), lhsO.astype(np.float16), bias


def _get_exec():
    """Build the Bass module and a cached jitted dispatch for it.

    Mirrors bass2jax.run_bass_via_pjrt, with two changes: the jitted callable
    is built once and reused (no per-call retrace), and the donated output
    buffer is created on-device by a cached zeros jit instead of being
    uploaded as host np.zeros (saves the full output size in H2D traffic).
    """
    bass2jax.install_neuronx_cc_hook()
    nc = _build_nc()
    assert nc.dbg_addr is None

    partition_name = nc.partition_id_tensor.name if nc.partition_id_tensor else None
    in_names: list[str] = []
    out_names: list[str] = []
    out_avals: list = []
    for alloc in nc.m.functions[0].allocations:
        if not isinstance(alloc, mybir.MemoryLocationSet):
            continue
        name = alloc.memorylocations[0].name
        if alloc.kind == "ExternalInput":
            if name != partition_name:
                in_names.append(name)
        elif alloc.kind == "ExternalOutput":
            shape = tuple(alloc.tensor_shape)
            dtype = mybir.dt.np(alloc.dtype)
            out_avals.append(jax.core.ShapedArray(shape, dtype))
            out_names.append(name)
    n_params = len(in_names)
    n_outs = len(out_avals)
    assert in_names == ["xpad", "rsc", "lhsE", "lhsO", "bias"] and n_outs == 1
    in_names = in_names + out_names
    if partition_name is not None:
        in_names.append(partition_name)

    def _body(*args):
        operands = list(args)
        if partition_name is not None:
            operands.append(bass2jax.partition_id_tensor())
        outs = bass2jax._bass_exec_p.bind(
            *operands,
            out_avals=tuple(out_avals),
            in_names=tuple(in_names),
            out_names=tuple(out_names),
            lowering_input_output_aliases=(),
            sim_require_finite=True,
            sim_require_nnan=True,
            nc=nc,
        )
        return tuple(outs)

    devices = jax.devices()[:N_CORES]
    assert len(devices) == N_CORES
    mesh = Mesh(np.asarray(devices), ("core",))
    in_specs = (PartitionSpec("core"),) * (n_params + n_outs)
    out_specs = (PartitionSpec("core"),) * n_outs
    donate = tuple(range(n_params, n_params + n_outs))
    sharded = jax.jit(
        shard_map(_body, mesh=mesh, in_specs=in_specs, out_specs=out_specs,
                  check_rep=False),
        donate_argnums=donate,
        keep_unused=True,
    )

    shard_sharding = NamedSharding(mesh, PartitionSpec("core"))
    zeros_fn = jax.jit(
        lambda: jnp.zeros((N_CORES * FLAT,), jnp.uint8),
        out_shardings=shard_sharding,
    )
    return sharded, zeros_fn, shard_sharding


def kernel(x, conv_w, conv_b, gamma, beta, run_mean, run_var, _trace=False):
    x = np.asarray(x, np.float32).reshape(B, H, W)
    conv_w = np.asarray(conv_w, np.float32)
    conv_b = np.asarray(conv_b, np.float32)
    gamma = np.asarray(gamma, np.float32)
    beta = np.asarray(beta, np.float32)
    run_mean = np.asarray(run_mean, np.float32)
    run_var = np.asarray(run_var, np.float32)

    if "exec" not in _CACHE:
        _CACHE["exec"] = _get_exec()
    sharded, zeros_fn, shard_sharding = _CACHE["exec"]

    # weights are tiny; upload once and reuse the device-resident copies
    wkey = (conv_w.tobytes(), conv_b.tobytes(), gamma.tobytes(), beta.tobytes(),
            run_mean.tobytes(), run_var.tobytes())
    cached = _CACHE.get("weights")
    if cached is None or cached[0] != wkey:
        lhsE, lhsO, bias = _host_weights(conv_w, conv_b, gamma, beta,
                                         run_mean, run_var)
        lE_d = jax.device_put(np.tile(lhsE, (N_CORES, 1)), shard_sharding)
        lO_d = jax.device_put(np.tile(lhsO, (N_CORES, 1)), shard_sharding)
        b_d = jax.device_put(np.tile(bias, (N_CORES, 1)), shard_sharding)
        _CACHE["weights"] = (wkey, lE_d, lO_d, b_d)
    _, lE_d, lO_d, b_d = _CACHE["weights"]

    x8 = x.reshape(N_CORES, PB, H, W)
    out = np.empty((N_CORES, PB, OC, HO, WO), np.float32)

    # Dispatch on a dedicated thread (sharded() blocks on its H2D), fetch on
    # a 2-thread pool: uploads of chunk n+1 stream while chunk n downloads.
    fetchq: queue.Queue = queue.Queue()
    with ThreadPoolExecutor(3) as ex:
        def dispatcher():
            for c in range(NCHUNK):
                # per-row-pair int8 quantization scales (shared per image pair)
                xc = x8[:, c * PBC:(c + 1) * PBC].reshape(N_CORES * PBC, H, W)
                rm = np.abs(xc).max(axis=2).reshape(-1, 2, H)   # [pairs, 2, H]
                rmp = rm.max(axis=1)                            # [pairs, H]
                s_row = np.maximum(rmp * np.float32(1.0 / 127.0),
                                   np.float32(1e-30))
                r_img = np.repeat(np.float32(1.0) / s_row, 2, axis=0)[:, :, None]
                s_pad = np.ones((s_row.shape[0], PH), np.float32)
                s_pad[:, 2:2 + H] = s_row
                sw = np.lib.stride_tricks.sliding_window_view(
                    s_pad, KROWS, axis=1)[:, ::YB]              # [pairs, NT, KROWS]
                rc = np.ascontiguousarray(np.broadcast_to(
                    sw[:, :, None, :], (s_row.shape[0], NT, 5, KROWS)
                )).reshape(N_CORES * NP * NT, K)
                xpad = np.zeros((N_CORES * PBC, PH, PW), np.int8)
                np.clip(np.rint(xc * r_img), -127, 127,
                        out=xpad[:, 2:2 + H, 2:2 + W], casting="unsafe")
                z = zeros_fn()
                (o,) = sharded(xpad, rc, lE_d, lO_d, b_d, z)
                fetchq.put((c, ex.submit(np.asarray, o)))

        disp = threading.Thread(target=dispatcher)
        disp.start()
        for _ in range(NCHUNK):
            c, fut = fetchq.get()
            flat = fut.result().reshape(N_CORES, FLAT)
            q = flat[:, :QBYTES]
            m = (
                flat[:, QBYTES:]
                .view(np.float32)
                .reshape(N_CORES, NP, NT, OC, 8)
            )
            m7 = (m.transpose(0, 1, 3, 2, 4)
                  .reshape(N_CORES, NP, 1, OC, NT, 8, 1)
                  * np.float32(1.0 / QMAX))
            dq = q.reshape(N_CORES, NP, 2, OC, NT, 8, WO).astype(np.float32) * m7
            out[:, c * PBC:(c + 1) * PBC] = dq.reshape(N_CORES, PBC, OC, HO, WO)
        disp.join()

    return np.ascontiguousarray(out.reshape(B, OC, HO, WO))


# revision 14
# speedup vs baseline: 1.0942x; 1.0942x over previous
"""Conv2d(1->16,5x5,p2) + BN(inference) + ReLU + MaxPool2d(2) on 8 NeuronCores.

The end-to-end call is bound by the axon tunnel (~30-40MB/s, full duplex), so
the design minimizes bytes on the wire and overlaps upload with download:
  - x is padded + cast to fp16 on the host (13.3MB up instead of 26.6MB f32).
  - Conv weights (BN-folded, Toeplitz lhsT) are fp16, uploaded once and cached
    on-device across calls.
  - The kernel quantizes each output slab to uint8 with a per-partition scale
    (the slab max, computed on-device). Scale f32 bytes are bitcast into the
    tail of the same u8 output buffer, so each chunk returns ONE tensor:
    ~6.4MB down per chunk instead of 12.9MB f32.
  - The batch is split into C=4 chunks pipelined through a cached jit: chunk
    n+1 uploads while chunk n downloads (full-duplex tunnel), fetches run on
    a 2-thread pool, dequantization overlaps the remaining transfers.
  - Donated output buffers are created on-device by a cached zeros jit, not
    uploaded from the host.

Device strategy (per core, data parallelism over batch):
  - BN is folded into the conv weights/bias on the host.
  - Conv is computed on the TensorEngine as a single matmul per 16-output-row
    slab: contraction K = (dx-block j in 0..4) x (input row yi in 0..19) = 100.
    The 5 dx shifts are materialized as 5 partition-blocks of the slab tile,
    loaded directly from HBM with column offset j (overlapping reads).
    The dy taps are encoded in a Toeplitz weight matrix lhsT[(j,yi), (o,yp)].
  - Two matmuls per slab produce even / odd output rows in separate PSUM
    banks, so the 2x2 maxpool becomes: vertical max = elementwise max of the
    two PSUM tiles (DVE), horizontal max = strided max in SBUF, then the
    quantization tail: per-partition slab max -> reciprocal -> fused
    ReLU+bias+scale activation casting straight to uint8.
"""

import numpy as np
import queue
import threading
from concurrent.futures import ThreadPoolExecutor

import jax
import jax.numpy as jnp
from jax.experimental.shard_map import shard_map
from jax.sharding import Mesh, NamedSharding, PartitionSpec

import concourse.bass as bass
import concourse.bacc as bacc
import concourse.tile as tile
import concourse.mybir as mybir
from concourse import bass2jax

F32 = mybir.dt.float32
F16 = mybir.dt.float16
U8 = mybir.dt.uint8
I8 = mybir.dt.int8
N_CORES = 8
B, H, W = 128, 224, 224
PB = B // N_CORES          # images per core
NCHUNK = 8                 # pipeline chunks per call
PBC = PB // NCHUNK         # images per core per chunk
NP = PBC // 2              # image pairs per core per chunk
PH, PW = H + 4, W + 4      # host-padded image
OC = 16
HO, WO = H // 2, W // 2    # 112, 112
YB = 16                    # conv output rows per slab
NT = H // YB               # 14 slabs per image pair
KROWS = YB + 4             # input rows per dx-block
K = 5 * KROWS              # 100 contraction partitions
BN_EPS = 1e-5
QMAX = 254.5               # u8 quantization full-scale (0.5 headroom for +0.5 rounding bias)
QBYTES = PBC * OC * HO * WO          # q payload bytes per core
SBYTES = NP * NT * 128 * 4           # scale payload bytes per core
FLAT = QBYTES + SBYTES

_CACHE: dict = {}


def _build_nc():
    nc = bacc.Bacc("TRN2", num_devices=N_CORES)
    xpad = nc.dram_tensor("xpad", [PBC, PH, PW], I8, kind="ExternalInput")
    rsc_d = nc.dram_tensor("rsc", [NP * NT, K], F32, kind="ExternalInput")
    lhsE_d = nc.dram_tensor("lhsE", [K, 128], F16, kind="ExternalInput")
    lhsO_d = nc.dram_tensor("lhsO", [K, 128], F16, kind="ExternalInput")
    bias_d = nc.dram_tensor("bias", [128, 1], F32, kind="ExternalInput")
    outb = nc.dram_tensor("outb", [FLAT], U8, kind="ExternalOutput")

    with tile.TileContext(nc) as tc:
        with (
            tc.tile_pool(name="const", bufs=1) as constp,
            tc.tile_pool(name="s", bufs=4) as sp,
            tc.tile_pool(name="v", bufs=3) as vp,
            tc.tile_pool(name="h", bufs=3) as hp,
            tc.tile_pool(name="m", bufs=3) as mp,
            tc.tile_pool(name="q", bufs=3) as qp,
            tc.tile_pool(name="ps", bufs=4, space="PSUM") as pp,
        ):
            lE = constp.tile([K, 128], F16, tag="lE")
            nc.sync.dma_start(lE[:], lhsE_d.ap())
            lO = constp.tile([K, 128], F16, tag="lO")
            nc.sync.dma_start(lO[:], lhsO_d.ap())
            bt = constp.tile([128, 1], F32, tag="bias")
            nc.sync.dma_start(bt[:], bias_d.ap())
            # per-slab per-partition input dequant scales: RS[p, col]
            RS = constp.tile([K, NP * NT], F32, tag="RS")
            nc.sync.dma_start(RS[:], bass.AP(rsc_d, 0, [[1, K], [K, NP * NT]]))
            # slab maxes, one column per (pair, slab); DMA'd out once at the end
            SC = constp.tile([128, NP * NT], F32, tag="SC")

            for pi in range(NP):            # image pairs
                for t in range(NT):         # y slabs
                    y0 = YB * t
                    col = pi * NT + t
                    S8 = sp.tile([K, 448], I8, tag="S8")
                    for i in range(2):
                        src = bass.AP(
                            xpad,
                            (2 * pi + i) * PH * PW + y0 * PW,
                            [[1, 5], [PW, KROWS], [1, 224]],
                        )
                        nc.sync.dma_start(S8[:, i * 224:(i + 1) * 224], src)
                    # dequant int8 -> fp16 with the per-row scale (ACT copy)
                    S = sp.tile([K, 448], F16, tag="S")
                    nc.scalar.activation(
                        S[:], S8[:], mybir.ActivationFunctionType.Copy,
                        scale=RS[:, col:col + 1],
                    )

                    pe_t = pp.tile([128, 448], F32, tag="ps")
                    nc.tensor.matmul(pe_t[:], lE[:], S[:], start=True, stop=True)
                    po_t = pp.tile([128, 448], F32, tag="ps")
                    nc.tensor.matmul(po_t[:], lO[:], S[:], start=True, stop=True)

                    # ACT drains the odd bank to SBUF (DVE cannot read two
                    # PSUM streams in one tensor_tensor)
                    CO = vp.tile([128, 448], F32, tag="CO")
                    nc.scalar.copy(CO[:], po_t[:])
                    # vertical max: PSUM + SBUF operands
                    V = vp.tile([128, 448], F32, tag="V")
                    nc.vector.tensor_max(V[:], pe_t[:], CO[:])
                    # horizontal max: strided SBUF
                    Hm = hp.tile([128, 224], F32, tag="H")
                    v4 = V[:].rearrange("p (i xp two) -> p i xp two", i=2, two=2)
                    h3 = Hm[:].rearrange("p (i xp) -> p i xp", i=2)
                    nc.vector.tensor_max(h3, v4[:, :, :, 0], v4[:, :, :, 1])

                    # ---- u8 quantization tail ----
                    Mx = mp.tile([128, 1], F32, tag="Mx")
                    nc.vector.reduce_max(Mx[:], Hm[:], axis=mybir.AxisListType.X)
                    # slab max of relu(conv+bias) = relu(max(conv)+bias)
                    Mr = mp.tile([128, 1], F32, tag="Mr")
                    nc.scalar.activation(
                        Mr[:], Mx[:], mybir.ActivationFunctionType.Relu,
                        bias=bt[:, 0:1], scale=1.0,
                    )
                    Mc = SC[:, col:col + 1]
                    nc.vector.tensor_scalar_max(Mc, Mr[:], 1e-20)
                    R = mp.tile([128, 1], F32, tag="R")
                    nc.vector.reciprocal(R[:], Mc)
                    Sg = mp.tile([128, 1], F32, tag="Sg")
                    nc.scalar.mul(Sg[:], R[:], QMAX)          # Sg = QMAX / max
                    # Bq = Sg*bias + 0.5 (0.5 turns the u8 trunc-cast into rounding)
                    Bq = mp.tile([128, 1], F32, tag="Bq")
                    nc.vector.tensor_scalar(
                        Bq[:], bt[:, 0:1], Sg[:, 0:1], 0.0,
                        op0=mybir.AluOpType.mult, op1=mybir.AluOpType.add,
                    )
                    # q = relu(Sg*conv + Sg*bias + 0.5) cast to u8
                    Q = qp.tile([128, 224], U8, tag="Q")
                    nc.scalar.activation(
                        Q[:], Hm[:], mybir.ActivationFunctionType.Relu,
                        bias=Bq[:, 0:1], scale=Sg[:, 0:1],
                    )

                    for i in range(2):
                        dst = bass.AP(
                            outb,
                            (2 * pi + i) * OC * HO * WO + (8 * t) * WO,
                            [[HO * WO, OC], [WO, 8], [1, WO]],
                        )
                        nc.scalar.dma_start(dst, Q[:, i * WO:(i + 1) * WO])

            # scale bytes: byte addr = QBYTES + (col*128 + m)*4 + k
            scb = SC[:].bitcast(U8).rearrange("p (c k) -> p c k", k=4)
            dst = bass.AP(outb, QBYTES, [[4, 128], [512, NP * NT], [1, 4]])
            nc.sync.dma_start(dst, scb)

    nc.compile()
    return nc


def _host_weights(conv_w, conv_b, gamma, beta, run_mean, run_var):
    scale = (gamma / np.sqrt(run_var + BN_EPS)).astype(np.float32)
    wf = (conv_w[:, 0] * scale[:, None, None]).astype(np.float32)       # [16,5,5]
    bf = (conv_b * scale + beta - run_mean * scale).astype(np.float32)  # [16]

    lhsE = np.zeros((K, 128), np.float32)
    lhsO = np.zeros((K, 128), np.float32)
    bias = np.zeros((128, 1), np.float32)
    for o in range(OC):
        for yp in range(8):
            m = o * 8 + yp
            bias[m, 0] = bf[o]
            for j in range(5):
                for dy in range(5):
                    lhsE[j * KROWS + 2 * yp + dy, m] = wf[o, dy, j]
                    lhsO[j * KROWS + 2 * yp + 1 + dy, m] = wf[o, dy, j]
    return lhsE.astype(np.float16), lhsO.astype(np.float16), bias


def _get_exec():
    """Build the Bass module and a cached jitted dispatch for it.

    Mirrors bass2jax.run_bass_via_pjrt, with two changes: the jitted callable
    is built once and reused (no per-call retrace), and the donated output
    buffer is created on-device by a cached zeros jit instead of being
    uploaded as host np.zeros (saves the full output size in H2D traffic).
    """
    bass2jax.install_neuronx_cc_hook()
    nc = _build_nc()
    assert nc.dbg_addr is None

    partition_name = nc.partition_id_tensor.name if nc.partition_id_tensor else None
    in_names: list[str] = []
    out_names: list[str] = []
    out_avals: list = []
    for alloc in nc.m.functions[0].allocations:
        if not isinstance(alloc, mybir.MemoryLocationSet):
            continue
        name = alloc.memorylocations[0].name
        if alloc.kind == "ExternalInput":
            if name != partition_name:
                in_names.append(name)
        elif alloc.kind == "ExternalOutput":
            shape = tuple(alloc.tensor_shape)
            dtype = mybir.dt.np(alloc.dtype)
            out_avals.append(jax.core.ShapedArray(shape, dtype))
            out_names.append(name)
    n_params = len(in_names)
    n_outs = len(out_avals)
    assert in_names == ["xpad", "rsc", "lhsE", "lhsO", "bias"] and n_outs == 1
    in_names = in_names + out_names
    if partition_name is not None:
        in_names.append(partition_name)

    def _body(*args):
        operands = list(args)
        if partition_name is not None:
            operands.append(bass2jax.partition_id_tensor())
        outs = bass2jax._bass_exec_p.bind(
            *operands,
            out_avals=tuple(out_avals),
            in_names=tuple(in_names),
            out_names=tuple(out_names),
            lowering_input_output_aliases=(),
            sim_require_finite=True,
            sim_require_nnan=True,
            nc=nc,
        )
        return tuple(outs)

    devices = jax.devices()[:N_CORES]
    assert len(devices) == N_CORES
    mesh = Mesh(np.asarray(devices), ("core",))
    in_specs = (PartitionSpec("core"),) * (n_params + n_outs)
    out_specs = (PartitionSpec("core"),) * n_outs
    donate = tuple(range(n_params, n_params + n_outs))
    sharded = jax.jit(
        shard_map(_body, mesh=mesh, in_specs=in_specs, out_specs=out_specs,
                  check_rep=False),
        donate_argnums=donate,
        keep_unused=True,
    )

    shard_sharding = NamedSharding(mesh, PartitionSpec("core"))
    zeros_fn = jax.jit(
        lambda: jnp.zeros((N_CORES * FLAT,), jnp.uint8),
        out_shardings=shard_sharding,
    )
    return sharded, zeros_fn, shard_sharding


def kernel(x, conv_w, conv_b, gamma, beta, run_mean, run_var, _trace=False):
    x = np.asarray(x, np.float32).reshape(B, H, W)
    conv_w = np.asarray(conv_w, np.float32)
    conv_b = np.asarray(conv_b, np.float32)
    gamma = np.asarray(gamma, np.float32)
    beta = np.asarray(beta, np.float32)
    run_mean = np.asarray(run_mean, np.float32)
    run_var = np.asarray(run_var, np.float32)

    if "exec" not in _CACHE:
        _CACHE["exec"] = _get_exec()
    sharded, zeros_fn, shard_sharding = _CACHE["exec"]

    # weights are tiny; upload once and reuse the device-resident copies
    wkey = (conv_w.tobytes(), conv_b.tobytes(), gamma.tobytes(), beta.tobytes(),
            run_mean.tobytes(), run_var.tobytes())
    cached = _CACHE.get("weights")
    if cached is None or cached[0] != wkey:
        lhsE, lhsO, bias = _host_weights(conv_w, conv_b, gamma, beta,
                                         run_mean, run_var)
        lE_d = jax.device_put(np.tile(lhsE, (N_CORES, 1)), shard_sharding)
        lO_d = jax.device_put(np.tile(lhsO, (N_CORES, 1)), shard_sharding)
        b_d = jax.device_put(np.tile(bias, (N_CORES, 1)), shard_sharding)
        _CACHE["weights"] = (wkey, lE_d, lO_d, b_d)
    _, lE_d, lO_d, b_d = _CACHE["weights"]

    x8 = x.reshape(N_CORES, PB, H, W)
    out = np.empty((N_CORES, PB, OC, HO, WO), np.float32)

    # Dispatch on a dedicated thread (sharded() blocks on its H2D), fetch on
    # a 2-thread pool: uploads of chunk n+1 stream while chunk n downloads.
    fetchq: queue.Queue = queue.Queue()
    with ThreadPoolExecutor(3) as ex:
        def dispatcher():
            for c in range(NCHUNK):
                # per-row-pair int8 quantization scales (shared per image pair)
                xc = x8[:, c * PBC:(c + 1) * PBC].reshape(N_CORES * PBC, H, W)
                rm = np.abs(xc).max(axis=2).reshape(-1, 2, H)   # [pairs, 2, H]
                rmp = rm.max(axis=1)                            # [pairs, H]
                s_row = np.maximum(rmp * np.float32(1.0 / 127.0),
                                   np.float32(1e-30))
                r_img = np.repeat(np.float32(1.0) / s_row, 2, axis=0)[:, :, None]
                s_pad = np.ones((s_row.shape[0], PH), np.float32)
                s_pad[:, 2:2 + H] = s_row
                sw = np.lib.stride_tricks.sliding_window_view(
                    s_pad, KROWS, axis=1)[:, ::YB]              # [pairs, NT, KROWS]
                rc = np.ascontiguousarray(np.broadcast_to(
                    sw[:, :, None, :], (s_row.shape[0], NT, 5, KROWS)
                )).reshape(N_CORES * NP * NT, K)
                xpad = np.zeros((N_CORES * PBC, PH, PW), np.int8)
                np.clip(np.rint(xc * r_img), -127, 127,
                        out=xpad[:, 2:2 + H, 2:2 + W], casting="unsafe")
                z = zeros_fn()
                (o,) = sharded(xpad, rc, lE_d, lO_d, b_d, z)
                try:
                    # queue D2H as soon as exec completes, ahead of the
                    # blocking np.asarray in the fetch worker
                    o.copy_to_host_async()
                except Exception:
                    pass
                fetchq.put((c, ex.submit(np.asarray, o)))

        disp = threading.Thread(target=dispatcher)
        disp.start()
        for _ in range(NCHUNK):
            c, fut = fetchq.get()
            flat = fut.result().reshape(N_CORES, FLAT)
            q = flat[:, :QBYTES]
            m = (
                flat[:, QBYTES:]
                .view(np.float32)
                .reshape(N_CORES, NP, NT, OC, 8)
            )
            m7 = (m.transpose(0, 1, 3, 2, 4)
                  .reshape(N_CORES, NP, 1, OC, NT, 8, 1)
                  * np.float32(1.0 / QMAX))
            dq = q.reshape(N_CORES, NP, 2, OC, NT, 8, WO).astype(np.float32) * m7
            out[:, c * PBC:(c + 1) * PBC] = dq.reshape(N_CORES, PBC, OC, HO, WO)
        disp.join()

    return np.ascontiguousarray(out.reshape(B, OC, HO, WO))
